# revision 19
# baseline (speedup 1.0000x reference)
"""Trainium2 Bass kernel for nn_Algebraic_interval: t-norm feature expansion.

For each input x in {xl, xu} of shape [65536, 16], computes
  out = concat([x, prod(x[:, idx2], -1), prod(x[:, idx3], -1)], axis=1)
over all C(16,2)=120 pair and C(16,3)=560 triple column combinations,
giving two [65536, 696] outputs (the harness tolerance is 2e-2, so the
device emits bf16 and the host widens to fp32).

Strategy (pure data parallel over 8 cores, 8192 rows each), transposed
layout: features in partitions, batch in the free dimension.  The PE
streams matmul columns at ~0.84 ns/col (fp32-PSUM write limited,
measured), so the design minimizes matmul passes (11 per chunk-column
sweep) and keeps every other engine under the DMA roofline:

  - lnx = ln(x + 1e-30) on ScalarE, emitted as fp32r (the fp32r matmul
    runs at full rate and keeps ~12 mantissa bits - plenty under bf16
    output rounding; no mantissa-split needed).
  - exp path (5 passes): pairs-l(120), pairs-u(120), and 352 "log"
    triples (3 blocks) via G-matmul of the logs + ScalarE exp.
  - mult path (6 passes): 768 triples in 6 blocks of 128.  Partition p
    has a FIXED largest column k(p) (exactly 3 pairs per partition and
    per half fit: 35x15 + 30x14 + 26x13 + 22x12 + 15x11 = 128
    partitions, 384 triples per half, zero waste).  Per block, TensorE
    one-hot-gathers the pair values into PSUM; VectorE multiplies by
    xrep (x replicated partition-wise, built once by 5 broadcast
    SBUF->SBUF DMAs per half) straight into the bf16 output slab.
  - singles (32 rows): host-provided bf16 x, DMA'd through.
  - DRAM output is macro-major [4, 1392, 2048] so every DMA lands in a
    sequential region (strided column-slices halve DMA efficiency).
    The host reorders rows/macros while transposing back to row-major.

Host-side: inputs are pre-transposed to feature-major xt[32, 8192]
fp32 (+ an xb bf16 copy); partition p<16: xl feature p; p>=16: xu.
"""

import itertools
import numpy as np

N_COLS = 16
B_FULL = 65536
N_CORES = 8
B_CORE = B_FULL // N_CORES          # 8192
PAIRS = list(itertools.combinations(range(N_COLS), 2))    # 120
TRIPLES = list(itertools.combinations(range(N_COLS), 3))  # 560
N_PAIR = len(PAIRS)
N_TRI = len(TRIPLES)
N_OUT = N_COLS + N_PAIR + N_TRI     # 696
PAIR_IDX = {p: i for i, p in enumerate(PAIRS)}

NC = 1024                            # pipeline chunk (PSUM tile width)
MACRO = 2048                         # DMA slab width (2 chunks)
N_CHUNK = B_CORE // NC               # 8
N_MACRO = B_CORE // MACRO            # 4

# mult-path packing: partition -> fixed k, 3 pairs (one per block/half)
K_ALLOC = [(15, 35), (14, 30), (13, 26), (12, 22), (11, 15)]  # (k, #parts)
N_MULT_BLK = 6                       # 3 l-blocks then 3 u-blocks


def _pack_mult():
    """Partition table (shared by both halves) + log-path leftovers.

    Returns (parts, log_pool): parts[p] = (k, [pair0, pair1, pair2]);
    log_pool = triples not covered (176 per half).
    """
    parts = []
    covered = set()
    for k, cnt in K_ALLOC:
        pk = list(itertools.combinations(range(k), 2))
        for t in range(cnt):
            trio = pk[3 * t : 3 * t + 3]
            parts.append((k, trio))
            for ij in trio:
                covered.add((ij[0], ij[1], k))
    assert len(parts) == 128
    log_pool = [t for t in TRIPLES if t not in covered]
    assert len(log_pool) == 176, len(log_pool)
    return parts, log_pool


_PARTS, _LOG_POOL = _pack_mult()
N_LOG = 2 * len(_LOG_POOL)           # 352
LOG_ROWS = [128, 128, N_LOG - 256]   # rows per log block (96 last)
N_LOG_BLK = 3

# device row layout (within one macro of outT)
ROW_SING = 0          # 32 rows: singles l(16) then u(16)
ROW_PAIR_L = 32       # 120 rows
ROW_PAIR_U = 152      # 120 rows
ROW_TRI = 272         # 6*128 mult rows then N_LOG log rows
ROW_LOG = ROW_TRI + N_MULT_BLK * 128
N_ROWS = ROW_LOG + N_LOG             # 1392

_CACHED = {}


def _make_mats():
    """Static matmul operands + host row maps.

    g2 [32, 240]  : log-sum matrix for the pair rows (fp32; cast to
                    fp32r on device).
    g3 [32, 352]  : log-sum matrix for the log-path triples.
    hp [120, 384] : one-hot pair gather, 3 blocks of 128 (shared by
                    the l and u mult blocks; bf16).
    dev_row[(half, tri)] -> device row index.
    """
    import ml_dtypes

    bf16 = ml_dtypes.bfloat16
    g2 = np.zeros((32, 2 * N_PAIR), dtype=np.float32)
    for half in (0, 1):
        for pi, (i, j) in enumerate(PAIRS):
            for f in (i, j):
                g2[half * 16 + f, half * N_PAIR + pi] = 1.0

    hp = np.zeros((N_PAIR, 3 * 128), dtype=np.float32)
    dev_row = {}
    for p, (k, trio) in enumerate(_PARTS):
        for b, (i, j) in enumerate(trio):
            hp[PAIR_IDX[(i, j)], b * 128 + p] = 1.0
            dev_row[(0, (i, j, k))] = ROW_TRI + b * 128 + p
            dev_row[(1, (i, j, k))] = ROW_TRI + (3 + b) * 128 + p

    g3 = np.zeros((32, N_LOG), dtype=np.float32)
    c = 0
    for half in (0, 1):
        for (i, j, k) in _LOG_POOL:
            for f in (i, j, k):
                g3[half * 16 + f, c] = 1.0
            dev_row[(half, (i, j, k))] = ROW_LOG + c
            c += 1
    assert c == N_LOG and len(dev_row) == 2 * N_TRI

    il = np.empty(N_OUT, dtype=np.int64)
    iu = np.empty(N_OUT, dtype=np.int64)
    for half, arr in ((0, il), (1, iu)):
        arr[0:N_COLS] = half * 16 + np.arange(16)
        arr[N_COLS : N_COLS + N_PAIR] = (
            (ROW_PAIR_L if half == 0 else ROW_PAIR_U) + np.arange(N_PAIR)
        )
        for t, tri in enumerate(TRIPLES):
            arr[N_COLS + N_PAIR + t] = dev_row[(half, tri)]
    return g2, g3, hp.astype(bf16), il, iu


def _build_program():
    import concourse.bacc as bacc
    import concourse.mybir as mybir
    import concourse.tile as tile
    from concourse.bass import MemorySpace

    f32 = mybir.dt.float32
    f32r = mybir.dt.float32r
    bf16 = mybir.dt.bfloat16
    Act = mybir.ActivationFunctionType
    nc = bacc.Bacc("TRN2", target_bir_lowering=False, debug=False)

    # const AP for the Ln bias (1e-30 is normal fp32, so no FTZ risk;
    # ln(0 + 1e-30) = -69.08 and exp of any sum including it underflows
    # to the (near-)exact 0 product)
    _c = nc.alloc_sbuf_tensor("const-float32-tiny", [128, 1], f32)
    nc.gpsimd.memset(_c.ap(), 1e-30)
    nc.const_aps.aps[(f32, 1e-30)] = _c.ap()

    xt = nc.dram_tensor("xt", [32, B_CORE], f32, kind="ExternalInput")
    xb = nc.dram_tensor("xb", [32, B_CORE], bf16, kind="ExternalInput")
    xrl = nc.dram_tensor("xrl", [128, B_CORE], bf16, kind="ExternalInput")
    xru = nc.dram_tensor("xru", [128, B_CORE], bf16, kind="ExternalInput")
    outT = nc.dram_tensor(
        "outT", [N_MACRO, N_ROWS, MACRO], bf16, kind="ExternalOutput"
    )
    g2_np, g3_np, hp_np, _, _ = _make_mats()
    g2 = nc.inline_tensor(g2_np, name="g2")
    g3 = nc.inline_tensor(g3_np, name="g3")
    hp = nc.inline_tensor(hp_np, name="hp")

    with tile.TileContext(nc) as tc:
        with (
            tc.tile_pool(name="const", bufs=1) as const_pool,
            tc.tile_pool(name="inp", bufs=1) as inp_pool,
            tc.tile_pool(name="scratch", bufs=2) as scratch_pool,
            tc.tile_pool(name="pairs", bufs=3) as pairs_pool,
            tc.tile_pool(name="slab", bufs=2) as slab_pool,
            tc.tile_pool(name="psum", bufs=4, space=MemorySpace.PSUM) as psum_pool,
        ):
            g2_f = const_pool.tile([32, 2 * N_PAIR], f32, tag="g2f")
            g3_f = const_pool.tile([32, N_LOG], f32, tag="g3f")
            hp_sb = const_pool.tile([N_PAIR, 3 * 128], bf16, tag="hp")
            nc.sync.dma_start(g2_f[:], g2[:])
            nc.sync.dma_start(g3_f[:], g3[:])
            nc.sync.dma_start(hp_sb[:], hp[:])
            # fp32r matmul operands must be produced pre-rounded; the
            # 0/1 selector entries are exact, so a one-time cast works.
            g2_sb = const_pool.tile([32, 2 * N_PAIR], f32r, tag="g2")
            nc.vector.tensor_copy(g2_sb[:], g2_f[:])
            g3_sb = const_pool.tile([32, N_LOG], f32r, tag="g3")
            nc.vector.tensor_copy(g3_sb[:], g3_f[:])

            # per-macro inputs; xt first (it gates the Ln prologue),
            # the mult-path xrep / xb inputs are staged just-in-time from
            # inside the main loop so they don't contend with xt or the
            # early pipeline at DMA-packet granularity.
            xt_sbs, xb_sbs, xrl_sbs, xru_sbs = [], [], [], []
            for m in range(N_MACRO):
                for h2_ in range(2):
                    c2 = 2 * m + h2_
                    xt_sb = inp_pool.tile(
                        [32, NC], f32, tag="xt_sb", name=f"xt{c2}"
                    )
                    nc.sync.dma_start(
                        xt_sb[:], xt[:, c2 * NC : (c2 + 1) * NC]
                    )
                    xt_sbs.append(xt_sb)
                xb_sbs.append(
                    inp_pool.tile([32, MACRO], bf16, tag="xb_sb", name=f"xb{m}")
                )
                xrl_sbs.append(
                    inp_pool.tile([128, MACRO], bf16, tag="xrl_sb", name=f"xrl{m}")
                )
                xru_sbs.append(
                    inp_pool.tile([128, MACRO], bf16, tag="xru_sb", name=f"xru{m}")
                )

            def stage_inputs(m):
                mcols = slice(m * MACRO, (m + 1) * MACRO)
                nc.sync.dma_start(xrl_sbs[m][:], xrl[:, mcols])
                nc.sync.dma_start(xru_sbs[m][:], xru[:, mcols])
                nc.sync.dma_start(xb_sbs[m][:], xb[:, mcols])

            stage_inputs(0)

            # Ln as two big instructions: the scheduler cannot interleave
            # them with Exp's, so the act-table set switches ~twice instead
            # of per-chunk (each load costs ~2.7us of ScalarE)
            lnx_half = []
            for g in range(2):
                lnxg = scratch_pool.tile(
                    [32, 4 * NC], f32r, tag="lnx", name=f"lnxg{g}"
                )
                lnx_half.append(lnxg)
            lnxs = []
            for c in range(N_CHUNK):
                g, q4 = divmod(c, 4)
                nc.scalar.activation(
                    lnx_half[g][:, q4 * NC : (q4 + 1) * NC],
                    xt_sbs[c][:],
                    Act.Ln,
                    bias=1e-30,
                )
                lnxs.append(lnx_half[g][:, q4 * NC : (q4 + 1) * NC])

            # ---- main pipeline (PE software-pipelined by one chunk:
            # the pair-gather matmuls of chunk c-1 are emitted after the
            # pairs/log matmuls of chunk c, so the PE never waits on the
            # pair exps) --------------------------------------------------
            state = {}   # per-chunk tiles needed one iteration later

            def emit_front(c):
                m, h = divmod(c, MACRO // NC)
                lnx = lnxs[c]

                hcols = slice(h * NC, (h + 1) * NC)
                if h == 0:
                    pl = pairs_pool.tile([N_PAIR, MACRO], bf16, tag="pl", name=f"pl{m}")
                    pu = pairs_pool.tile([N_PAIR, MACRO], bf16, tag="pu", name=f"pu{m}")
                    slab = slab_pool.tile([128, 9, MACRO], bf16, tag="slab", name=f"slab{m}")
                    state[m] = (pl, pu, slab)
                pl, pu, slab = state[m]

                def mm2(ps, rows, lhsT):
                    for q in (0, 1):
                        nc.tensor.matmul(
                            ps[0:rows, q * 512 : (q + 1) * 512],
                            lhsT,
                            lnx[:, q * 512 : (q + 1) * 512],
                        )

                ps_l = psum_pool.tile([128, NC], f32, tag="ps", name=f"psl{c}")
                mm2(ps_l, N_PAIR, g2_sb[:, 0:N_PAIR])
                ps_u = psum_pool.tile([128, NC], f32, tag="ps", name=f"psu{c}")
                mm2(ps_u, N_PAIR, g2_sb[:, N_PAIR : 2 * N_PAIR])
                nc.scalar.activation(pl[:, hcols], ps_l[0:N_PAIR, :], Act.Exp)
                nc.scalar.activation(pu[:, hcols], ps_u[0:N_PAIR, :], Act.Exp)
                r0 = 0
                for lb in range(N_LOG_BLK):
                    rows = LOG_ROWS[lb]
                    ps = psum_pool.tile([128, NC], f32, tag="ps", name=f"pslog{c}_{lb}")
                    mm2(ps, rows, g3_sb[:, r0 : r0 + rows])
                    nc.scalar.activation(
                        slab[0:rows, N_MULT_BLK + lb, hcols],
                        ps[0:rows, :],
                        Act.Exp,
                    )
                    r0 += rows

            def emit_back(c):
                m, h = divmod(c, MACRO // NC)
                pl, pu, slab = state[m]
                hcols = slice(h * NC, (h + 1) * NC)
                for b in range(N_MULT_BLK):
                    src_t = pl if b < 3 else pu
                    xrep = xrl_sbs[m] if b < 3 else xru_sbs[m]
                    o = (b % 3) * 128
                    ps_pg = psum_pool.tile([128, NC], f32, tag="ps", name=f"pspg{c}_{b}")
                    for q in (0, 1):
                        nc.tensor.matmul(
                            ps_pg[:, q * 512 : (q + 1) * 512],
                            hp_sb[:, o : o + 128],
                            src_t[:, h * NC + q * 512 : h * NC + (q + 1) * 512],
                        )
                    nc.vector.tensor_mul(
                        slab[:, b, hcols], ps_pg[:], xrep[:, hcols]
                    )
                # per-chunk output DMAs: halves the end-of-kernel DMA
                # drain and smooths DMA engine load
                nc.sync.dma_start(
                    outT[m, ROW_SING : ROW_SING + 32, hcols],
                    xb_sbs[m][:, hcols],
                )
                nc.sync.dma_start(
                    outT[m, ROW_PAIR_L : ROW_PAIR_L + N_PAIR, hcols],
                    pl[:, hcols],
                )
                nc.sync.dma_start(
                    outT[m, ROW_PAIR_U : ROW_PAIR_U + N_PAIR, hcols],
                    pu[:, hcols],
                )
                ot = outT.ap()[m, ROW_TRI : ROW_TRI + 8 * 128, hcols]
                nc.gpsimd.dma_start(
                    ot.rearrange("(b p) c -> p b c", p=128),
                    slab[:, 0:8, hcols],
                )
                nc.gpsimd.dma_start(
                    outT[m, ROW_TRI + 8 * 128 : N_ROWS, hcols],
                    slab[0 : LOG_ROWS[2], 8, hcols],
                )

            emit_front(0)
            stage_inputs(1)
            emit_front(1)
            for c in range(2, N_CHUNK):
                emit_front(c)
                if c % 2 == 0 and c // 2 + 1 < N_MACRO:
                    stage_inputs(c // 2 + 1)
                emit_back(c - 2)
            emit_back(N_CHUNK - 2)
            emit_back(N_CHUNK - 1)

    nc.compile()
    return nc


def _spot_check(xl, xu, full_l, full_u, n_rows=48) -> bool:
    """Validate sampled rows against an exact host-side recomputation."""
    if not (np.isfinite(full_l).all() and np.isfinite(full_u).all()):
        return False
    rows = np.linspace(0, B_FULL - 1, n_rows, dtype=np.int64)
    idx2 = np.array(PAIRS)
    idx3 = np.array(TRIPLES)
    for x, out in ((xl, full_l), (xu, full_u)):
        xs = x[rows].astype(np.float64)
        exp = np.concatenate(
            [xs, np.prod(xs[:, idx2], -1), np.prod(xs[:, idx3], -1)], axis=1
        )
        rel = np.abs(out[rows] - exp) / np.maximum(np.abs(exp), 1e-9)
        if rel.max() > 1.5e-2:
            return False
    return True


def kernel(xl, xu):
    from concourse.bass_utils import run_bass_kernel_spmd

    xl = np.asarray(xl, dtype=np.float32)
    xu = np.asarray(xu, dtype=np.float32)

    if "nc" not in _CACHED:
        _CACHED["nc"] = _build_program()
    nc = _CACHED["nc"]

    import ml_dtypes

    kmap = np.concatenate(
        [np.full(cnt, k, dtype=np.int64) for k, cnt in K_ALLOC]
    )
    in_maps = []
    for i in range(N_CORES):
        lo, hi = i * B_CORE, (i + 1) * B_CORE
        xt = np.ascontiguousarray(
            np.concatenate([xl[lo:hi].T, xu[lo:hi].T], axis=0)
        )
        xbv = xt.astype(ml_dtypes.bfloat16)
        in_maps.append({
            "xt": xt,
            "xb": xbv,
            "xrl": np.ascontiguousarray(xbv[kmap]),
            "xru": np.ascontiguousarray(xbv[16 + kmap]),
        })

    *_, il, iu = _make_mats()
    # retry loop: guards against rare transient device/DMA corruption
    last_err = None
    full_l = full_u = None
    for attempt in range(3):
        try:
            res = run_bass_kernel_spmd(nc, in_maps, list(range(N_CORES)))
        except Exception as e:  # transient device error: retry
            last_err = e
            import time

            time.sleep(3)
            continue
        full_l = np.empty((B_FULL, N_OUT), dtype=np.float32)
        full_u = np.empty((B_FULL, N_OUT), dtype=np.float32)
        for i in range(N_CORES):
            lo, hi = i * B_CORE, (i + 1) * B_CORE
            ot = res.results[i]["outT"]            # [4, N_ROWS, MACRO]
            ot = ot.transpose(1, 0, 2).reshape(N_ROWS, B_CORE)
            full_l[lo:hi] = ot[il].T
            full_u[lo:hi] = ot[iu].T
        if _spot_check(xl, xu, full_l, full_u):
            return full_l, full_u
    if full_l is None:
        raise last_err
    return full_l, full_u


# revision 20
# speedup vs baseline: 1.0905x; 1.0905x over previous
"""Trainium2 Bass kernel for nn_Algebraic_interval: t-norm feature expansion.

For each input x in {xl, xu} of shape [65536, 16], computes
  out = concat([x, prod(x[:, idx2], -1), prod(x[:, idx3], -1)], axis=1)
over all C(16,2)=120 pair and C(16,3)=560 triple column combinations,
giving two [65536, 696] outputs (the harness tolerance is 2e-2, so the
device emits bf16 and the host widens to fp32).

Strategy (pure data parallel over 8 cores, 8192 rows each), transposed
layout: features in partitions, batch in the free dimension.  The PE
streams matmul columns at ~0.84 ns/col (fp32-PSUM write limited,
measured), so the design minimizes matmul passes (11 per chunk-column
sweep) and keeps every other engine under the DMA roofline:

  - lnx = ln(x + 1e-30) on ScalarE, emitted as fp32r (the fp32r matmul
    runs at full rate and keeps ~12 mantissa bits - plenty under bf16
    output rounding; no mantissa-split needed).
  - exp path (5 passes): pairs-l(120), pairs-u(120), and 352 "log"
    triples (3 blocks) via G-matmul of the logs + ScalarE exp.
  - mult path (6 passes): 768 triples in 6 blocks of 128.  Partition p
    has a FIXED largest column k(p) (exactly 3 pairs per partition and
    per half fit: 35x15 + 30x14 + 26x13 + 22x12 + 15x11 = 128
    partitions, 384 triples per half, zero waste).  Per block, TensorE
    one-hot-gathers the pair values into PSUM; VectorE multiplies by
    xrep (x replicated partition-wise, built once by 5 broadcast
    SBUF->SBUF DMAs per half) straight into the bf16 output slab.
  - singles (32 rows): host-provided bf16 x, DMA'd through.
  - DRAM output is macro-major [4, 1392, 2048] so every DMA lands in a
    sequential region (strided column-slices halve DMA efficiency).
    The host reorders rows/macros while transposing back to row-major.

Host-side: inputs are pre-transposed to feature-major xt[32, 8192]
fp32 (+ an xb bf16 copy); partition p<16: xl feature p; p>=16: xu.
"""

import itertools
import numpy as np

N_COLS = 16
B_FULL = 65536
N_CORES = 8
B_CORE = B_FULL // N_CORES          # 8192
PAIRS = list(itertools.combinations(range(N_COLS), 2))    # 120
TRIPLES = list(itertools.combinations(range(N_COLS), 3))  # 560
N_PAIR = len(PAIRS)
N_TRI = len(TRIPLES)
N_OUT = N_COLS + N_PAIR + N_TRI     # 696
PAIR_IDX = {p: i for i, p in enumerate(PAIRS)}

NC = 1024                            # pipeline chunk (PSUM tile width)
MACRO = 2048                         # DMA slab width (2 chunks)
N_CHUNK = B_CORE // NC               # 8
N_MACRO = B_CORE // MACRO            # 4

# mult-path packing: partition -> fixed k, 3 pairs (one per block/half)
K_ALLOC = [(15, 35), (14, 30), (13, 26), (12, 22), (11, 15)]  # (k, #parts)
N_MULT_BLK = 6                       # 3 l-blocks then 3 u-blocks


def _pack_mult():
    """Partition table (shared by both halves) + log-path leftovers.

    Returns (parts, log_pool): parts[p] = (k, [pair0, pair1, pair2]);
    log_pool = triples not covered (176 per half).
    """
    parts = []
    covered = set()
    for k, cnt in K_ALLOC:
        pk = list(itertools.combinations(range(k), 2))
        for t in range(cnt):
            trio = pk[3 * t : 3 * t + 3]
            parts.append((k, trio))
            for ij in trio:
                covered.add((ij[0], ij[1], k))
    assert len(parts) == 128
    log_pool = [t for t in TRIPLES if t not in covered]
    assert len(log_pool) == 176, len(log_pool)
    return parts, log_pool


_PARTS, _LOG_POOL = _pack_mult()
N_LOG = 2 * len(_LOG_POOL)           # 352
LOG_ROWS = [128, 128, N_LOG - 256]   # rows per log block (96 last)
N_LOG_BLK = 3

# device row layout (within one macro of outT)
ROW_SING = 0          # 32 rows: singles l(16) then u(16)
ROW_PAIR_L = 32       # 120 rows
ROW_PAIR_U = 152      # 120 rows
ROW_TRI = 272         # 6*128 mult rows then N_LOG log rows
ROW_LOG = ROW_TRI + N_MULT_BLK * 128
N_ROWS = ROW_LOG + N_LOG             # 1392

_CACHED = {}


def _make_mats():
    """Static matmul operands + host row maps.

    g2 [32, 240]  : log-sum matrix for the pair rows (fp32; cast to
                    fp32r on device).
    g3 [32, 352]  : log-sum matrix for the log-path triples.
    hp [120, 384] : one-hot pair gather, 3 blocks of 128 (shared by
                    the l and u mult blocks; bf16).
    dev_row[(half, tri)] -> device row index.
    """
    import ml_dtypes

    bf16 = ml_dtypes.bfloat16
    g2 = np.zeros((32, 2 * N_PAIR), dtype=np.float32)
    for half in (0, 1):
        for pi, (i, j) in enumerate(PAIRS):
            for f in (i, j):
                g2[half * 16 + f, half * N_PAIR + pi] = 1.0

    hp = np.zeros((N_PAIR, 3 * 128), dtype=np.float32)
    dev_row = {}
    for p, (k, trio) in enumerate(_PARTS):
        for b, (i, j) in enumerate(trio):
            hp[PAIR_IDX[(i, j)], b * 128 + p] = 1.0
            dev_row[(0, (i, j, k))] = ROW_TRI + b * 128 + p
            dev_row[(1, (i, j, k))] = ROW_TRI + (3 + b) * 128 + p

    g3 = np.zeros((32, N_LOG), dtype=np.float32)
    c = 0
    for half in (0, 1):
        for (i, j, k) in _LOG_POOL:
            for f in (i, j, k):
                g3[half * 16 + f, c] = 1.0
            dev_row[(half, (i, j, k))] = ROW_LOG + c
            c += 1
    assert c == N_LOG and len(dev_row) == 2 * N_TRI

    il = np.empty(N_OUT, dtype=np.int64)
    iu = np.empty(N_OUT, dtype=np.int64)
    for half, arr in ((0, il), (1, iu)):
        arr[0:N_COLS] = half * 16 + np.arange(16)
        arr[N_COLS : N_COLS + N_PAIR] = (
            (ROW_PAIR_L if half == 0 else ROW_PAIR_U) + np.arange(N_PAIR)
        )
        for t, tri in enumerate(TRIPLES):
            arr[N_COLS + N_PAIR + t] = dev_row[(half, tri)]
    return g2, g3, hp.astype(bf16), il, iu


def _build_program():
    import concourse.bacc as bacc
    import concourse.mybir as mybir
    import concourse.tile as tile
    from concourse.bass import MemorySpace

    f32 = mybir.dt.float32
    f32r = mybir.dt.float32r
    bf16 = mybir.dt.bfloat16
    Act = mybir.ActivationFunctionType
    nc = bacc.Bacc("TRN2", target_bir_lowering=False, debug=False)

    # const AP for the Ln bias (1e-30 is normal fp32, so no FTZ risk;
    # ln(0 + 1e-30) = -69.08 and exp of any sum including it underflows
    # to the (near-)exact 0 product)
    _c = nc.alloc_sbuf_tensor("const-float32-tiny", [128, 1], f32)
    nc.gpsimd.memset(_c.ap(), 1e-30)
    nc.const_aps.aps[(f32, 1e-30)] = _c.ap()

    xt = nc.dram_tensor("xt", [32, B_CORE], f32, kind="ExternalInput")
    xb = nc.dram_tensor("xb", [32, B_CORE], bf16, kind="ExternalInput")
    xrl = nc.dram_tensor("xrl", [128, B_CORE], bf16, kind="ExternalInput")
    xru = nc.dram_tensor("xru", [128, B_CORE], bf16, kind="ExternalInput")
    outT = nc.dram_tensor(
        "outT", [N_MACRO, N_ROWS, MACRO], bf16, kind="ExternalOutput"
    )
    g2_np, g3_np, hp_np, _, _ = _make_mats()
    g2 = nc.inline_tensor(g2_np, name="g2")
    g3 = nc.inline_tensor(g3_np, name="g3")
    hp = nc.inline_tensor(hp_np, name="hp")

    with tile.TileContext(nc) as tc:
        with (
            tc.tile_pool(name="const", bufs=1) as const_pool,
            tc.tile_pool(name="inp", bufs=1) as inp_pool,
            tc.tile_pool(name="scratch", bufs=8) as scratch_pool,
            tc.tile_pool(name="pairs", bufs=3) as pairs_pool,
            tc.tile_pool(name="slab", bufs=2) as slab_pool,
            tc.tile_pool(name="psum", bufs=4, space=MemorySpace.PSUM) as psum_pool,
        ):
            g2_f = const_pool.tile([32, 2 * N_PAIR], f32, tag="g2f")
            g3_f = const_pool.tile([32, N_LOG], f32, tag="g3f")
            hp_sb = const_pool.tile([N_PAIR, 3 * 128], bf16, tag="hp")
            nc.sync.dma_start(g2_f[:], g2[:])
            nc.sync.dma_start(g3_f[:], g3[:])
            nc.sync.dma_start(hp_sb[:], hp[:])
            # fp32r matmul operands must be produced pre-rounded; the
            # 0/1 selector entries are exact, so a one-time cast works.
            g2_sb = const_pool.tile([32, 2 * N_PAIR], f32r, tag="g2")
            nc.vector.tensor_copy(g2_sb[:], g2_f[:])
            g3_sb = const_pool.tile([32, N_LOG], f32r, tag="g3")
            nc.vector.tensor_copy(g3_sb[:], g3_f[:])

            # per-macro inputs; xt first (it gates the Ln prologue),
            # the mult-path xrep / xb inputs are staged just-in-time from
            # inside the main loop so they don't contend with xt or the
            # early pipeline at DMA-packet granularity.
            xt_sbs, xb_sbs, xrl_sbs, xru_sbs = [], [], [], []
            for m in range(N_MACRO):
                for h2_ in range(2):
                    c2 = 2 * m + h2_
                    xt_sb = inp_pool.tile(
                        [32, NC], f32, tag="xt_sb", name=f"xt{c2}"
                    )
                    nc.sync.dma_start(
                        xt_sb[:], xt[:, c2 * NC : (c2 + 1) * NC]
                    )
                    xt_sbs.append(xt_sb)
                xb_sbs.append(
                    inp_pool.tile([32, MACRO], bf16, tag="xb_sb", name=f"xb{m}")
                )
                xrl_sbs.append(
                    inp_pool.tile([128, MACRO], bf16, tag="xrl_sb", name=f"xrl{m}")
                )
                xru_sbs.append(
                    inp_pool.tile([128, MACRO], bf16, tag="xru_sb", name=f"xru{m}")
                )

            def stage_inputs(m):
                mcols = slice(m * MACRO, (m + 1) * MACRO)
                nc.sync.dma_start(xrl_sbs[m][:], xrl[:, mcols])
                nc.sync.dma_start(xru_sbs[m][:], xru[:, mcols])
                nc.sync.dma_start(xb_sbs[m][:], xb[:, mcols])

            stage_inputs(0)

            lnxs = []
            for c in range(N_CHUNK):
                lnx = scratch_pool.tile([32, NC], f32r, tag="lnx", name=f"lnx{c}")
                nc.scalar.activation(
                    lnx[:], xt_sbs[c][:], Act.Ln, bias=1e-30
                )
                lnxs.append(lnx)

            # ---- main pipeline --------------------------------------
            # Software-pipelined by one chunk AND interleaved at PSUM-tile
            # level: tiles drained by ScalarE (pair/log exps of chunk c)
            # alternate with tiles drained by VectorE (pair-gathers of
            # chunk c-1), so the 4-slot PSUM ring always has both consumer
            # engines pulling and the PE never waits on a single engine.
            state = {}

            def make_macro(m):
                pl = pairs_pool.tile([N_PAIR, MACRO], bf16, tag="pl", name=f"pl{m}")
                pu = pairs_pool.tile([N_PAIR, MACRO], bf16, tag="pu", name=f"pu{m}")
                slab = slab_pool.tile([128, 9, MACRO], bf16, tag="slab", name=f"slab{m}")
                state[m] = (pl, pu, slab)

            def front_units(c):
                """(kind, emit) producers of chunk c consumed by ScalarE."""
                m, h = divmod(c, MACRO // NC)
                lnx = lnxs[c]
                hcols = slice(h * NC, (h + 1) * NC)
                if h == 0:
                    make_macro(m)
                pl, pu, slab = state[m]

                def mm2(ps, rows, lhsT):
                    for q in (0, 1):
                        nc.tensor.matmul(
                            ps[0:rows, q * 512 : (q + 1) * 512],
                            lhsT,
                            lnx[:, q * 512 : (q + 1) * 512],
                        )

                def u_pair(which):
                    def emit():
                        ps = psum_pool.tile([128, NC], f32, tag="ps",
                                            name=f"ps{which}{c}")
                        if which == "l":
                            mm2(ps, N_PAIR, g2_sb[:, 0:N_PAIR])
                            nc.scalar.activation(
                                pl[:, hcols], ps[0:N_PAIR, :], Act.Exp)
                        else:
                            mm2(ps, N_PAIR, g2_sb[:, N_PAIR : 2 * N_PAIR])
                            nc.scalar.activation(
                                pu[:, hcols], ps[0:N_PAIR, :], Act.Exp)
                    return emit

                def u_log(lb, r0):
                    def emit():
                        rows = LOG_ROWS[lb]
                        ps = psum_pool.tile([128, NC], f32, tag="ps",
                                            name=f"pslog{c}_{lb}")
                        mm2(ps, rows, g3_sb[:, r0 : r0 + rows])
                        nc.scalar.activation(
                            slab[0:rows, N_MULT_BLK + lb, hcols],
                            ps[0:rows, :], Act.Exp)
                    return emit

                units = [u_pair("l"), u_pair("u")]
                r0 = 0
                for lb in range(N_LOG_BLK):
                    units.append(u_log(lb, r0))
                    r0 += LOG_ROWS[lb]
                return units

            def back_units(c):
                """(kind, emit) pair-gather+mult units of chunk c (DVE)."""
                m, h = divmod(c, MACRO // NC)
                pl, pu, slab = state[m]
                hcols = slice(h * NC, (h + 1) * NC)

                def u_mult(b):
                    def emit():
                        src_t = pl if b < 3 else pu
                        xrep = xrl_sbs[m] if b < 3 else xru_sbs[m]
                        o = (b % 3) * 128
                        ps_pg = psum_pool.tile([128, NC], f32, tag="ps",
                                               name=f"pspg{c}_{b}")
                        for q in (0, 1):
                            nc.tensor.matmul(
                                ps_pg[:, q * 512 : (q + 1) * 512],
                                hp_sb[:, o : o + 128],
                                src_t[:, h * NC + q * 512
                                      : h * NC + (q + 1) * 512],
                            )
                        nc.vector.tensor_mul(
                            slab[:, b, hcols], ps_pg[:], xrep[:, hcols])
                    return emit

                return [u_mult(b) for b in range(N_MULT_BLK)]

            def emit_dmas(c):
                m, h = divmod(c, MACRO // NC)
                pl, pu, slab = state[m]
                hcols = slice(h * NC, (h + 1) * NC)
                nc.sync.dma_start(
                    outT[m, ROW_SING : ROW_SING + 32, hcols],
                    xb_sbs[m][:, hcols])
                nc.sync.dma_start(
                    outT[m, ROW_PAIR_L : ROW_PAIR_L + N_PAIR, hcols],
                    pl[:, hcols])
                nc.sync.dma_start(
                    outT[m, ROW_PAIR_U : ROW_PAIR_U + N_PAIR, hcols],
                    pu[:, hcols])
                ot = outT.ap()[m, ROW_TRI : ROW_TRI + 8 * 128, hcols]
                nc.gpsimd.dma_start(
                    ot.rearrange("(b p) c -> p b c", p=128),
                    slab[:, 0:8, hcols])
                nc.gpsimd.dma_start(
                    outT[m, ROW_TRI + 8 * 128 : N_ROWS, hcols],
                    slab[0 : LOG_ROWS[2], 8, hcols])

            def interleave(f_units, b_units):
                out = []
                fi = bi = 0
                while fi < len(f_units) or bi < len(b_units):
                    if fi < len(f_units):
                        out.append(f_units[fi]); fi += 1
                    if bi < len(b_units):
                        out.append(b_units[bi]); bi += 1
                return out

            prev_back = []
            for c in range(N_CHUNK):
                for u in interleave(front_units(c), prev_back):
                    u()
                if c >= 1:
                    emit_dmas(c - 1)
                prev_back = back_units(c)
                if c % 2 == 1 and c // 2 + 1 < N_MACRO:
                    stage_inputs(c // 2 + 1)
            for u in prev_back:
                u()
            emit_dmas(N_CHUNK - 1)

    nc.compile()
    return nc


def _spot_check(xl, xu, full_l, full_u, n_rows=48) -> bool:
    """Validate sampled rows against an exact host-side recomputation."""
    if not (np.isfinite(full_l).all() and np.isfinite(full_u).all()):
        return False
    rows = np.linspace(0, B_FULL - 1, n_rows, dtype=np.int64)
    idx2 = np.array(PAIRS)
    idx3 = np.array(TRIPLES)
    for x, out in ((xl, full_l), (xu, full_u)):
        xs = x[rows].astype(np.float64)
        exp = np.concatenate(
            [xs, np.prod(xs[:, idx2], -1), np.prod(xs[:, idx3], -1)], axis=1
        )
        rel = np.abs(out[rows] - exp) / np.maximum(np.abs(exp), 1e-9)
        if rel.max() > 1.5e-2:
            return False
    return True


def kernel(xl, xu):
    from concourse.bass_utils import run_bass_kernel_spmd

    xl = np.asarray(xl, dtype=np.float32)
    xu = np.asarray(xu, dtype=np.float32)

    if "nc" not in _CACHED:
        _CACHED["nc"] = _build_program()
    nc = _CACHED["nc"]

    import ml_dtypes

    kmap = np.concatenate(
        [np.full(cnt, k, dtype=np.int64) for k, cnt in K_ALLOC]
    )
    in_maps = []
    for i in range(N_CORES):
        lo, hi = i * B_CORE, (i + 1) * B_CORE
        xt = np.ascontiguousarray(
            np.concatenate([xl[lo:hi].T, xu[lo:hi].T], axis=0)
        )
        xbv = xt.astype(ml_dtypes.bfloat16)
        in_maps.append({
            "xt": xt,
            "xb": xbv,
            "xrl": np.ascontiguousarray(xbv[kmap]),
            "xru": np.ascontiguousarray(xbv[16 + kmap]),
        })

    *_, il, iu = _make_mats()
    # retry loop: guards against rare transient device/DMA corruption
    last_err = None
    full_l = full_u = None
    for attempt in range(3):
        try:
            res = run_bass_kernel_spmd(nc, in_maps, list(range(N_CORES)))
        except Exception as e:  # transient device error: retry
            last_err = e
            import time

            time.sleep(3)
            continue
        full_l = np.empty((B_FULL, N_OUT), dtype=np.float32)
        full_u = np.empty((B_FULL, N_OUT), dtype=np.float32)
        for i in range(N_CORES):
            lo, hi = i * B_CORE, (i + 1) * B_CORE
            ot = res.results[i]["outT"]            # [4, N_ROWS, MACRO]
            ot = ot.transpose(1, 0, 2).reshape(N_ROWS, B_CORE)
            full_l[lo:hi] = ot[il].T
            full_u[lo:hi] = ot[iu].T
        if _spot_check(xl, xu, full_l, full_u):
            return full_l, full_u
    if full_l is None:
        raise last_err
    return full_l, full_u


# revision 21
# speedup vs baseline: 1.1037x; 1.0122x over previous
"""Trainium2 Bass kernel for nn_Algebraic_interval: t-norm feature expansion.

For each input x in {xl, xu} of shape [65536, 16], computes
  out = concat([x, prod(x[:, idx2], -1), prod(x[:, idx3], -1)], axis=1)
over all C(16,2)=120 pair and C(16,3)=560 triple column combinations,
giving two [65536, 696] outputs (the harness tolerance is 2e-2, so the
device emits bf16 and the host widens to fp32).

Strategy (pure data parallel over 8 cores, 8192 rows each), transposed
layout: features in partitions, batch in the free dimension.  The PE
streams matmul columns at ~0.84 ns/col (fp32-PSUM write limited,
measured), so the design minimizes matmul passes (11 per chunk-column
sweep) and keeps every other engine under the DMA roofline:

  - lnx = ln(x + 1e-30) on ScalarE, emitted as fp32r (the fp32r matmul
    runs at full rate and keeps ~12 mantissa bits - plenty under bf16
    output rounding; no mantissa-split needed).
  - exp path (5 passes): pairs-l(120), pairs-u(120), and 352 "log"
    triples (3 blocks) via G-matmul of the logs + ScalarE exp.
  - mult path (6 passes): 768 triples in 6 blocks of 128.  Partition p
    has a FIXED largest column k(p) (exactly 3 pairs per partition and
    per half fit: 35x15 + 30x14 + 26x13 + 22x12 + 15x11 = 128
    partitions, 384 triples per half, zero waste).  Per block, TensorE
    one-hot-gathers the pair values into PSUM; VectorE multiplies by
    xrep (x replicated partition-wise, built once by 5 broadcast
    SBUF->SBUF DMAs per half) straight into the bf16 output slab.
  - singles (32 rows): host-provided bf16 x, DMA'd through.
  - DRAM output is macro-major [4, 1392, 2048] so every DMA lands in a
    sequential region (strided column-slices halve DMA efficiency).
    The host reorders rows/macros while transposing back to row-major.

Host-side: inputs are pre-transposed to feature-major xt[32, 8192]
fp32 (+ an xb bf16 copy); partition p<16: xl feature p; p>=16: xu.
"""

import itertools
import numpy as np

N_COLS = 16
B_FULL = 65536
N_CORES = 8
B_CORE = B_FULL // N_CORES          # 8192
PAIRS = list(itertools.combinations(range(N_COLS), 2))    # 120
TRIPLES = list(itertools.combinations(range(N_COLS), 3))  # 560
N_PAIR = len(PAIRS)
N_TRI = len(TRIPLES)
N_OUT = N_COLS + N_PAIR + N_TRI     # 696
PAIR_IDX = {p: i for i, p in enumerate(PAIRS)}

NC = 1024                            # pipeline chunk (PSUM tile width)
MACRO = 2048                         # DMA slab width (2 chunks)
N_CHUNK = B_CORE // NC               # 8
N_MACRO = B_CORE // MACRO            # 4

# mult-path packing: partition -> fixed k, 3 pairs (one per block/half)
K_ALLOC = [(15, 35), (14, 30), (13, 26), (12, 22), (11, 15)]  # (k, #parts)
N_MULT_BLK = 6                       # 3 l-blocks then 3 u-blocks


def _pack_mult():
    """Partition table (shared by both halves) + log-path leftovers.

    Returns (parts, log_pool): parts[p] = (k, [pair0, pair1, pair2]);
    log_pool = triples not covered (176 per half).
    """
    parts = []
    covered = set()
    for k, cnt in K_ALLOC:
        pk = list(itertools.combinations(range(k), 2))
        for t in range(cnt):
            trio = pk[3 * t : 3 * t + 3]
            parts.append((k, trio))
            for ij in trio:
                covered.add((ij[0], ij[1], k))
    assert len(parts) == 128
    log_pool = [t for t in TRIPLES if t not in covered]
    assert len(log_pool) == 176, len(log_pool)
    return parts, log_pool


_PARTS, _LOG_POOL = _pack_mult()
N_LOG = 2 * len(_LOG_POOL)           # 352
LOG_ROWS = [128, 128, N_LOG - 256]   # rows per log block (96 last)
N_LOG_BLK = 3

# device row layout (within one macro of outT)
ROW_SING = 0          # 32 rows: singles l(16) then u(16)
ROW_PAIR_L = 32       # 120 rows
ROW_PAIR_U = 152      # 120 rows
ROW_TRI = 272         # 6*128 mult rows then N_LOG log rows
ROW_LOG = ROW_TRI + N_MULT_BLK * 128
N_ROWS = ROW_LOG + N_LOG             # 1392

_CACHED = {}


def _make_mats():
    """Static matmul operands + host row maps.

    g2 [32, 240]  : log-sum matrix for the pair rows (fp32; cast to
                    fp32r on device).
    g3 [32, 352]  : log-sum matrix for the log-path triples.
    hp [120, 384] : one-hot pair gather, 3 blocks of 128 (shared by
                    the l and u mult blocks; bf16).
    dev_row[(half, tri)] -> device row index.
    """
    import ml_dtypes

    bf16 = ml_dtypes.bfloat16
    g2 = np.zeros((32, 2 * N_PAIR), dtype=np.float32)
    for half in (0, 1):
        for pi, (i, j) in enumerate(PAIRS):
            for f in (i, j):
                g2[half * 16 + f, half * N_PAIR + pi] = 1.0

    hp = np.zeros((N_PAIR, 3 * 128), dtype=np.float32)
    dev_row = {}
    for p, (k, trio) in enumerate(_PARTS):
        for b, (i, j) in enumerate(trio):
            hp[PAIR_IDX[(i, j)], b * 128 + p] = 1.0
            dev_row[(0, (i, j, k))] = ROW_TRI + b * 128 + p
            dev_row[(1, (i, j, k))] = ROW_TRI + (3 + b) * 128 + p

    g3 = np.zeros((32, N_LOG), dtype=np.float32)
    c = 0
    for half in (0, 1):
        for (i, j, k) in _LOG_POOL:
            for f in (i, j, k):
                g3[half * 16 + f, c] = 1.0
            dev_row[(half, (i, j, k))] = ROW_LOG + c
            c += 1
    assert c == N_LOG and len(dev_row) == 2 * N_TRI

    il = np.empty(N_OUT, dtype=np.int64)
    iu = np.empty(N_OUT, dtype=np.int64)
    for half, arr in ((0, il), (1, iu)):
        arr[0:N_COLS] = half * 16 + np.arange(16)
        arr[N_COLS : N_COLS + N_PAIR] = (
            (ROW_PAIR_L if half == 0 else ROW_PAIR_U) + np.arange(N_PAIR)
        )
        for t, tri in enumerate(TRIPLES):
            arr[N_COLS + N_PAIR + t] = dev_row[(half, tri)]
    return g2, g3, hp.astype(bf16), il, iu


def _build_program():
    import concourse.bacc as bacc
    import concourse.mybir as mybir
    import concourse.tile as tile
    from concourse.bass import MemorySpace

    f32 = mybir.dt.float32
    f32r = mybir.dt.float32r
    bf16 = mybir.dt.bfloat16
    Act = mybir.ActivationFunctionType
    nc = bacc.Bacc("TRN2", target_bir_lowering=False, debug=False)

    # const AP for the Ln bias (1e-30 is normal fp32, so no FTZ risk;
    # ln(0 + 1e-30) = -69.08 and exp of any sum including it underflows
    # to the (near-)exact 0 product)
    _c = nc.alloc_sbuf_tensor("const-float32-tiny", [128, 1], f32)
    nc.gpsimd.memset(_c.ap(), 1e-30)
    nc.const_aps.aps[(f32, 1e-30)] = _c.ap()

    xt = nc.dram_tensor("xt", [32, B_CORE], f32, kind="ExternalInput")
    xb = nc.dram_tensor("xb", [32, B_CORE], bf16, kind="ExternalInput")
    xrl = nc.dram_tensor("xrl", [128, B_CORE], bf16, kind="ExternalInput")
    xru = nc.dram_tensor("xru", [128, B_CORE], bf16, kind="ExternalInput")
    outT = nc.dram_tensor(
        "outT", [N_MACRO, N_ROWS, MACRO], bf16, kind="ExternalOutput"
    )
    g2_np, g3_np, hp_np, _, _ = _make_mats()
    g2 = nc.inline_tensor(g2_np, name="g2")
    g3 = nc.inline_tensor(g3_np, name="g3")
    hp = nc.inline_tensor(hp_np, name="hp")

    with tile.TileContext(nc) as tc:
        with (
            tc.tile_pool(name="const", bufs=1) as const_pool,
            tc.tile_pool(name="inp", bufs=1) as inp_pool,
            tc.tile_pool(name="scratch", bufs=8) as scratch_pool,
            tc.tile_pool(name="pairs", bufs=3) as pairs_pool,
            tc.tile_pool(name="slab", bufs=2) as slab_pool,
            tc.tile_pool(name="psum", bufs=4, space=MemorySpace.PSUM) as psum_pool,
        ):
            g2_f = const_pool.tile([32, 2 * N_PAIR], f32, tag="g2f")
            g3_f = const_pool.tile([32, N_LOG], f32, tag="g3f")
            hp_sb = const_pool.tile([N_PAIR, 3 * 128], bf16, tag="hp")
            nc.sync.dma_start(g2_f[:], g2[:])
            nc.sync.dma_start(g3_f[:], g3[:])
            nc.sync.dma_start(hp_sb[:], hp[:])
            # fp32r matmul operands must be produced pre-rounded; the
            # 0/1 selector entries are exact, so a one-time cast works.
            g2_sb = const_pool.tile([32, 2 * N_PAIR], f32r, tag="g2")
            nc.vector.tensor_copy(g2_sb[:], g2_f[:])
            g3_sb = const_pool.tile([32, N_LOG], f32r, tag="g3")
            nc.vector.tensor_copy(g3_sb[:], g3_f[:])

            # per-macro inputs; xt first (it gates the Ln prologue),
            # the mult-path xrep / xb inputs are staged just-in-time from
            # inside the main loop so they don't contend with xt or the
            # early pipeline at DMA-packet granularity.
            xt_sbs, xb_sbs, xrl_sbs, xru_sbs = [], [], [], []
            for m in range(N_MACRO):
                for h2_ in range(2):
                    c2 = 2 * m + h2_
                    xt_sb = inp_pool.tile(
                        [32, NC], f32, tag="xt_sb", name=f"xt{c2}"
                    )
                    nc.sync.dma_start(
                        xt_sb[:], xt[:, c2 * NC : (c2 + 1) * NC]
                    )
                    xt_sbs.append(xt_sb)
                xb_sbs.append(
                    inp_pool.tile([32, MACRO], bf16, tag="xb_sb", name=f"xb{m}")
                )
                xrl_sbs.append(
                    inp_pool.tile([128, MACRO], bf16, tag="xrl_sb", name=f"xrl{m}")
                )
                xru_sbs.append(
                    inp_pool.tile([128, MACRO], bf16, tag="xru_sb", name=f"xru{m}")
                )

            def stage_inputs(m):
                mcols = slice(m * MACRO, (m + 1) * MACRO)
                nc.sync.dma_start(xrl_sbs[m][:], xrl[:, mcols])
                nc.sync.dma_start(xru_sbs[m][:], xru[:, mcols])
                nc.sync.dma_start(xb_sbs[m][:], xb[:, mcols])

            # Ln's at scheduler priority 0: they run as soon as their xt
            # chunk lands, back to back, so the act table set switches
            # once instead of per chunk.  The xrep/xb inputs are staged
            # after the Ln's so the xt chunks win the DMA bandwidth race.
            lnxs = []
            with tc.high_priority():
                for c in range(N_CHUNK):
                    lnx = scratch_pool.tile(
                        [32, NC], f32r, tag="lnx", name=f"lnx{c}"
                    )
                    nc.scalar.activation(
                        lnx[:], xt_sbs[c][:], Act.Ln, bias=1e-30
                    )
                    lnxs.append(lnx)
            stage_inputs(0)

            # ---- main pipeline --------------------------------------
            # Software-pipelined by one chunk AND interleaved at PSUM-tile
            # level: tiles drained by ScalarE (pair/log exps of chunk c)
            # alternate with tiles drained by VectorE (pair-gathers of
            # chunk c-1), so the 4-slot PSUM ring always has both consumer
            # engines pulling and the PE never waits on a single engine.
            state = {}

            def make_macro(m):
                pl = pairs_pool.tile([N_PAIR, MACRO], bf16, tag="pl", name=f"pl{m}")
                pu = pairs_pool.tile([N_PAIR, MACRO], bf16, tag="pu", name=f"pu{m}")
                slab = slab_pool.tile([128, 9, MACRO], bf16, tag="slab", name=f"slab{m}")
                state[m] = (pl, pu, slab)

            def front_units(c):
                """(kind, emit) producers of chunk c consumed by ScalarE."""
                m, h = divmod(c, MACRO // NC)
                lnx = lnxs[c]
                hcols = slice(h * NC, (h + 1) * NC)
                if h == 0:
                    make_macro(m)
                pl, pu, slab = state[m]

                def mm2(ps, rows, lhsT):
                    for q in (0, 1):
                        nc.tensor.matmul(
                            ps[0:rows, q * 512 : (q + 1) * 512],
                            lhsT,
                            lnx[:, q * 512 : (q + 1) * 512],
                        )

                def u_pair(which):
                    def emit():
                        ps = psum_pool.tile([128, NC], f32, tag="ps",
                                            name=f"ps{which}{c}")
                        if which == "l":
                            mm2(ps, N_PAIR, g2_sb[:, 0:N_PAIR])
                            nc.scalar.activation(
                                pl[:, hcols], ps[0:N_PAIR, :], Act.Exp)
                        else:
                            mm2(ps, N_PAIR, g2_sb[:, N_PAIR : 2 * N_PAIR])
                            nc.scalar.activation(
                                pu[:, hcols], ps[0:N_PAIR, :], Act.Exp)
                    return emit

                def u_log(lb, r0):
                    def emit():
                        rows = LOG_ROWS[lb]
                        ps = psum_pool.tile([128, NC], f32, tag="ps",
                                            name=f"pslog{c}_{lb}")
                        mm2(ps, rows, g3_sb[:, r0 : r0 + rows])
                        nc.scalar.activation(
                            slab[0:rows, N_MULT_BLK + lb, hcols],
                            ps[0:rows, :], Act.Exp)
                    return emit

                units = [u_pair("l"), u_pair("u")]
                r0 = 0
                for lb in range(N_LOG_BLK):
                    units.append(u_log(lb, r0))
                    r0 += LOG_ROWS[lb]
                return units

            def back_units(c):
                """(kind, emit) pair-gather+mult units of chunk c (DVE)."""
                m, h = divmod(c, MACRO // NC)
                pl, pu, slab = state[m]
                hcols = slice(h * NC, (h + 1) * NC)

                def u_mult(b):
                    def emit():
                        src_t = pl if b < 3 else pu
                        xrep = xrl_sbs[m] if b < 3 else xru_sbs[m]
                        o = (b % 3) * 128
                        ps_pg = psum_pool.tile([128, NC], f32, tag="ps",
                                               name=f"pspg{c}_{b}")
                        for q in (0, 1):
                            nc.tensor.matmul(
                                ps_pg[:, q * 512 : (q + 1) * 512],
                                hp_sb[:, o : o + 128],
                                src_t[:, h * NC + q * 512
                                      : h * NC + (q + 1) * 512],
                            )
                        nc.vector.tensor_mul(
                            slab[:, b, hcols], ps_pg[:], xrep[:, hcols])
                    return emit

                return [u_mult(b) for b in range(N_MULT_BLK)]

            def emit_dmas(c, fine=False):
                m, h = divmod(c, MACRO // NC)
                pl, pu, slab = state[m]
                hcols = slice(h * NC, (h + 1) * NC)
                nc.sync.dma_start(
                    outT[m, ROW_SING : ROW_SING + 32, hcols],
                    xb_sbs[m][:, hcols])
                nc.sync.dma_start(
                    outT[m, ROW_PAIR_L : ROW_PAIR_L + N_PAIR, hcols],
                    pl[:, hcols])
                nc.sync.dma_start(
                    outT[m, ROW_PAIR_U : ROW_PAIR_U + N_PAIR, hcols],
                    pu[:, hcols])
                if not fine:
                    ot = outT.ap()[m, ROW_TRI : ROW_TRI + 8 * 128, hcols]
                    nc.gpsimd.dma_start(
                        ot.rearrange("(b p) c -> p b c", p=128),
                        slab[:, 0:8, hcols])
                    nc.gpsimd.dma_start(
                        outT[m, ROW_TRI + 8 * 128 : N_ROWS, hcols],
                        slab[0 : LOG_ROWS[2], 8, hcols])
                    return
                # per-block slab DMAs: each block streams out right after
                # its producer finishes -> short end-of-kernel drain
                for b in range(8):
                    nc.gpsimd.dma_start(
                        outT[m, ROW_TRI + b * 128 : ROW_TRI + (b + 1) * 128,
                             hcols],
                        slab[:, b, hcols])
                nc.gpsimd.dma_start(
                    outT[m, ROW_TRI + 8 * 128 : N_ROWS, hcols],
                    slab[0 : LOG_ROWS[2], 8, hcols])

            def interleave(f_units, b_units):
                out = []
                fi = bi = 0
                while fi < len(f_units) or bi < len(b_units):
                    if fi < len(f_units):
                        out.append(f_units[fi]); fi += 1
                    if bi < len(b_units):
                        out.append(b_units[bi]); bi += 1
                return out

            prev_back = []
            for c in range(N_CHUNK):
                for u in interleave(front_units(c), prev_back):
                    u()
                if c >= 1:
                    emit_dmas(c - 1)
                prev_back = back_units(c)
                if c % 2 == 1 and c // 2 + 1 < N_MACRO:
                    stage_inputs(c // 2 + 1)
            for u in prev_back:
                u()
            emit_dmas(N_CHUNK - 1, fine=True)

    nc.compile()
    return nc


def _spot_check(xl, xu, full_l, full_u, n_rows=48) -> bool:
    """Validate sampled rows against an exact host-side recomputation."""
    if not (np.isfinite(full_l).all() and np.isfinite(full_u).all()):
        return False
    rows = np.linspace(0, B_FULL - 1, n_rows, dtype=np.int64)
    idx2 = np.array(PAIRS)
    idx3 = np.array(TRIPLES)
    for x, out in ((xl, full_l), (xu, full_u)):
        xs = x[rows].astype(np.float64)
        exp = np.concatenate(
            [xs, np.prod(xs[:, idx2], -1), np.prod(xs[:, idx3], -1)], axis=1
        )
        rel = np.abs(out[rows] - exp) / np.maximum(np.abs(exp), 1e-9)
        if rel.max() > 1.5e-2:
            return False
    return True


def kernel(xl, xu):
    from concourse.bass_utils import run_bass_kernel_spmd

    xl = np.asarray(xl, dtype=np.float32)
    xu = np.asarray(xu, dtype=np.float32)

    if "nc" not in _CACHED:
        _CACHED["nc"] = _build_program()
    nc = _CACHED["nc"]

    import ml_dtypes

    kmap = np.concatenate(
        [np.full(cnt, k, dtype=np.int64) for k, cnt in K_ALLOC]
    )
    in_maps = []
    for i in range(N_CORES):
        lo, hi = i * B_CORE, (i + 1) * B_CORE
        xt = np.ascontiguousarray(
            np.concatenate([xl[lo:hi].T, xu[lo:hi].T], axis=0)
        )
        xbv = xt.astype(ml_dtypes.bfloat16)
        in_maps.append({
            "xt": xt,
            "xb": xbv,
            "xrl": np.ascontiguousarray(xbv[kmap]),
            "xru": np.ascontiguousarray(xbv[16 + kmap]),
        })

    *_, il, iu = _make_mats()
    # retry loop: guards against rare transient device/DMA corruption
    last_err = None
    full_l = full_u = None
    for attempt in range(3):
        try:
            res = run_bass_kernel_spmd(nc, in_maps, list(range(N_CORES)))
        except Exception as e:  # transient device error: retry
            last_err = e
            import time

            time.sleep(3)
            continue
        full_l = np.empty((B_FULL, N_OUT), dtype=np.float32)
        full_u = np.empty((B_FULL, N_OUT), dtype=np.float32)
        for i in range(N_CORES):
            lo, hi = i * B_CORE, (i + 1) * B_CORE
            ot = res.results[i]["outT"]            # [4, N_ROWS, MACRO]
            ot = ot.transpose(1, 0, 2).reshape(N_ROWS, B_CORE)
            full_l[lo:hi] = ot[il].T
            full_u[lo:hi] = ot[iu].T
        if _spot_check(xl, xu, full_l, full_u):
            return full_l, full_u
    if full_l is None:
        raise last_err
    return full_l, full_u


# revision 22
# speedup vs baseline: 1.1403x; 1.0332x over previous
"""Trainium2 Bass kernel for nn_Algebraic_interval: t-norm feature expansion.

For each input x in {xl, xu} of shape [65536, 16], computes
  out = concat([x, prod(x[:, idx2], -1), prod(x[:, idx3], -1)], axis=1)
over all C(16,2)=120 pair and C(16,3)=560 triple column combinations,
giving two [65536, 696] outputs (the harness tolerance is 2e-2, so the
device emits bf16 and the host widens to fp32).

Strategy (pure data parallel over 8 cores, 8192 rows each), transposed
layout: features in partitions, batch in the free dimension.  The PE
streams matmul columns at ~0.84 ns/col (fp32-PSUM write limited,
measured), so the design minimizes matmul passes (11 per chunk-column
sweep) and keeps every other engine under the DMA roofline:

  - lnx = ln(x + 1e-30) on ScalarE, emitted as fp32r (the fp32r matmul
    runs at full rate and keeps ~12 mantissa bits - plenty under bf16
    output rounding; no mantissa-split needed).
  - exp path (5 passes): pairs-l(120), pairs-u(120), and 352 "log"
    triples (3 blocks) via G-matmul of the logs + ScalarE exp.
  - mult path (6 passes): 768 triples in 6 blocks of 128.  Partition p
    has a FIXED largest column k(p) (exactly 3 pairs per partition and
    per half fit: 35x15 + 30x14 + 26x13 + 22x12 + 15x11 = 128
    partitions, 384 triples per half, zero waste).  Per block, TensorE
    one-hot-gathers the pair values into PSUM; VectorE multiplies by
    xrep (x replicated partition-wise, built once by 5 broadcast
    SBUF->SBUF DMAs per half) straight into the bf16 output slab.
  - singles (32 rows): host-provided bf16 x, DMA'd through.
  - DRAM output is macro-major [4, 1392, 2048] so every DMA lands in a
    sequential region (strided column-slices halve DMA efficiency).
    The host reorders rows/macros while transposing back to row-major.

Host-side: inputs are pre-transposed to feature-major xt[32, 8192]
fp32 (+ an xb bf16 copy); partition p<16: xl feature p; p>=16: xu.
"""

import itertools
import numpy as np

N_COLS = 16
B_FULL = 65536
N_CORES = 8
B_CORE = B_FULL // N_CORES          # 8192
PAIRS = list(itertools.combinations(range(N_COLS), 2))    # 120
TRIPLES = list(itertools.combinations(range(N_COLS), 3))  # 560
N_PAIR = len(PAIRS)
N_TRI = len(TRIPLES)
N_OUT = N_COLS + N_PAIR + N_TRI     # 696
PAIR_IDX = {p: i for i, p in enumerate(PAIRS)}

NC = 1024                            # pipeline chunk (PSUM tile width)
MACRO = 2048                         # DMA slab width (2 chunks)
N_CHUNK = B_CORE // NC               # 8
N_MACRO = B_CORE // MACRO            # 4

# mult-path packing: partition -> fixed k, 3 pairs (one per block/half)
K_ALLOC = [(15, 35), (14, 30), (13, 26), (12, 22), (11, 15)]  # (k, #parts)
N_MULT_BLK = 6                       # 3 l-blocks then 3 u-blocks


def _pack_mult():
    """Partition table (shared by both halves) + log-path leftovers.

    Returns (parts, log_pool): parts[p] = (k, [pair0, pair1, pair2]);
    log_pool = triples not covered (176 per half).
    """
    parts = []
    covered = set()
    for k, cnt in K_ALLOC:
        pk = list(itertools.combinations(range(k), 2))
        for t in range(cnt):
            trio = pk[3 * t : 3 * t + 3]
            parts.append((k, trio))
            for ij in trio:
                covered.add((ij[0], ij[1], k))
    assert len(parts) == 128
    log_pool = [t for t in TRIPLES if t not in covered]
    assert len(log_pool) == 176, len(log_pool)
    return parts, log_pool


_PARTS, _LOG_POOL = _pack_mult()
N_LOG = 2 * len(_LOG_POOL)           # 352
LOG_ROWS = [128, 128, N_LOG - 256]   # rows per log block (96 last)
N_LOG_BLK = 3

# device row layout (within one macro of outT)
ROW_SING = 0          # 32 rows: singles l(16) then u(16)
ROW_PAIR_L = 32       # 120 rows
ROW_PAIR_U = 152      # 120 rows
ROW_TRI = 272         # 6*128 mult rows then N_LOG log rows
ROW_LOG = ROW_TRI + N_MULT_BLK * 128
N_ROWS = ROW_LOG + N_LOG             # 1392

_CACHED = {}


def _make_mats():
    """Static matmul operands + host row maps.

    g2 [32, 240]  : log-sum matrix for the pair rows (fp32; cast to
                    fp32r on device).
    g3 [32, 352]  : log-sum matrix for the log-path triples.
    hp [120, 384] : one-hot pair gather, 3 blocks of 128 (shared by
                    the l and u mult blocks; bf16).
    dev_row[(half, tri)] -> device row index.
    """
    import ml_dtypes

    bf16 = ml_dtypes.bfloat16
    g2 = np.zeros((32, 2 * N_PAIR), dtype=np.float32)
    for half in (0, 1):
        for pi, (i, j) in enumerate(PAIRS):
            for f in (i, j):
                g2[half * 16 + f, half * N_PAIR + pi] = 1.0

    hp = np.zeros((N_PAIR, 3 * 128), dtype=np.float32)
    dev_row = {}
    for p, (k, trio) in enumerate(_PARTS):
        for b, (i, j) in enumerate(trio):
            hp[PAIR_IDX[(i, j)], b * 128 + p] = 1.0
            dev_row[(0, (i, j, k))] = ROW_TRI + b * 128 + p
            dev_row[(1, (i, j, k))] = ROW_TRI + (3 + b) * 128 + p

    g3 = np.zeros((32, N_LOG), dtype=np.float32)
    c = 0
    for half in (0, 1):
        for (i, j, k) in _LOG_POOL:
            for f in (i, j, k):
                g3[half * 16 + f, c] = 1.0
            dev_row[(half, (i, j, k))] = ROW_LOG + c
            c += 1
    assert c == N_LOG and len(dev_row) == 2 * N_TRI

    il = np.empty(N_OUT, dtype=np.int64)
    iu = np.empty(N_OUT, dtype=np.int64)
    for half, arr in ((0, il), (1, iu)):
        arr[0:N_COLS] = half * 16 + np.arange(16)
        arr[N_COLS : N_COLS + N_PAIR] = (
            (ROW_PAIR_L if half == 0 else ROW_PAIR_U) + np.arange(N_PAIR)
        )
        for t, tri in enumerate(TRIPLES):
            arr[N_COLS + N_PAIR + t] = dev_row[(half, tri)]
    return g2, g3, hp.astype(bf16), il, iu


def _build_program():
    import concourse.bacc as bacc
    import concourse.mybir as mybir
    import concourse.tile as tile
    from concourse.bass import MemorySpace

    f32 = mybir.dt.float32
    f32r = mybir.dt.float32r
    bf16 = mybir.dt.bfloat16
    Act = mybir.ActivationFunctionType
    nc = bacc.Bacc("TRN2", target_bir_lowering=False, debug=False)

    # const AP for the Ln bias (1e-30 is normal fp32, so no FTZ risk;
    # ln(0 + 1e-30) = -69.08 and exp of any sum including it underflows
    # to the (near-)exact 0 product)
    _c = nc.alloc_sbuf_tensor("const-float32-tiny", [128, 1], f32)
    nc.gpsimd.memset(_c.ap(), 1e-30)
    nc.const_aps.aps[(f32, 1e-30)] = _c.ap()

    xt = nc.dram_tensor("xt", [32, B_CORE], f32, kind="ExternalInput")
    xb = nc.dram_tensor("xb", [32, B_CORE], bf16, kind="ExternalInput")
    xrl = nc.dram_tensor("xrl", [128, B_CORE], bf16, kind="ExternalInput")
    xru = nc.dram_tensor("xru", [128, B_CORE], bf16, kind="ExternalInput")
    outT = nc.dram_tensor(
        "outT", [N_MACRO, N_ROWS, MACRO], bf16, kind="ExternalOutput"
    )
    g2_np, g3_np, hp_np, _, _ = _make_mats()
    g2 = nc.inline_tensor(g2_np, name="g2")
    g3 = nc.inline_tensor(g3_np, name="g3")
    hp = nc.inline_tensor(hp_np, name="hp")

    with tile.TileContext(nc) as tc:
        with (
            tc.tile_pool(name="const", bufs=1) as const_pool,
            tc.tile_pool(name="inp", bufs=1) as inp_pool,
            tc.tile_pool(name="scratch", bufs=8) as scratch_pool,
            tc.tile_pool(name="pairs", bufs=3) as pairs_pool,
            tc.tile_pool(name="slab", bufs=2) as slab_pool,
            tc.tile_pool(name="psumS", bufs=2, space=MemorySpace.PSUM) as psum_s,
            tc.tile_pool(name="psumV", bufs=2, space=MemorySpace.PSUM) as psum_v,
        ):
            g2_f = const_pool.tile([32, 2 * N_PAIR], f32, tag="g2f")
            g3_f = const_pool.tile([32, N_LOG], f32, tag="g3f")
            hp_sb = const_pool.tile([N_PAIR, 3 * 128], bf16, tag="hp")
            nc.sync.dma_start(g2_f[:], g2[:])
            nc.sync.dma_start(g3_f[:], g3[:])
            nc.sync.dma_start(hp_sb[:], hp[:])
            # fp32r matmul operands must be produced pre-rounded; the
            # 0/1 selector entries are exact, so a one-time cast works.
            g2_sb = const_pool.tile([32, 2 * N_PAIR], f32r, tag="g2")
            nc.vector.tensor_copy(g2_sb[:], g2_f[:])
            g3_sb = const_pool.tile([32, N_LOG], f32r, tag="g3")
            nc.vector.tensor_copy(g3_sb[:], g3_f[:])

            # per-macro inputs; xt first (it gates the Ln prologue),
            # the mult-path xrep / xb inputs are staged just-in-time from
            # inside the main loop so they don't contend with xt or the
            # early pipeline at DMA-packet granularity.
            xt_sbs, xb_sbs, xrl_sbs, xru_sbs = [], [], [], []
            for m in range(N_MACRO):
                for h2_ in range(2):
                    c2 = 2 * m + h2_
                    xt_sb = inp_pool.tile(
                        [32, NC], f32, tag="xt_sb", name=f"xt{c2}"
                    )
                    nc.sync.dma_start(
                        xt_sb[:], xt[:, c2 * NC : (c2 + 1) * NC]
                    )
                    xt_sbs.append(xt_sb)
                xb_sbs.append(
                    inp_pool.tile([32, MACRO], bf16, tag="xb_sb", name=f"xb{m}")
                )
                xrl_sbs.append(
                    inp_pool.tile([128, MACRO], bf16, tag="xrl_sb", name=f"xrl{m}")
                )
                xru_sbs.append(
                    inp_pool.tile([128, MACRO], bf16, tag="xru_sb", name=f"xru{m}")
                )

            def stage_inputs(m):
                mcols = slice(m * MACRO, (m + 1) * MACRO)
                nc.sync.dma_start(xrl_sbs[m][:], xrl[:, mcols])
                nc.sync.dma_start(xru_sbs[m][:], xru[:, mcols])
                nc.sync.dma_start(xb_sbs[m][:], xb[:, mcols])

            # Ln's at scheduler priority 0: they run as soon as their xt
            # chunk lands, back to back, so the act table set switches
            # once instead of per chunk.  The xrep/xb inputs are staged
            # after the Ln's so the xt chunks win the DMA bandwidth race.
            lnxs = []
            with tc.high_priority():
                for c in range(N_CHUNK):
                    lnx = scratch_pool.tile(
                        [32, NC], f32r, tag="lnx", name=f"lnx{c}"
                    )
                    nc.scalar.activation(
                        lnx[:], xt_sbs[c][:], Act.Ln, bias=1e-30
                    )
                    lnxs.append(lnx)
            stage_inputs(0)

            # ---- main pipeline --------------------------------------
            # Software-pipelined by one chunk AND interleaved at PSUM-tile
            # level: tiles drained by ScalarE (pair/log exps of chunk c)
            # alternate with tiles drained by VectorE (pair-gathers of
            # chunk c-1), so the 4-slot PSUM ring always has both consumer
            # engines pulling and the PE never waits on a single engine.
            state = {}

            def make_macro(m):
                pl = pairs_pool.tile([N_PAIR, MACRO], bf16, tag="pl", name=f"pl{m}")
                pu = pairs_pool.tile([N_PAIR, MACRO], bf16, tag="pu", name=f"pu{m}")
                slab = slab_pool.tile([128, 9, MACRO], bf16, tag="slab", name=f"slab{m}")
                state[m] = (pl, pu, slab)

            def front_units(c):
                """(kind, emit) producers of chunk c consumed by ScalarE."""
                m, h = divmod(c, MACRO // NC)
                lnx = lnxs[c]
                hcols = slice(h * NC, (h + 1) * NC)
                if h == 0:
                    make_macro(m)
                pl, pu, slab = state[m]

                def mm2(ps, rows, lhsT):
                    for q in (0, 1):
                        nc.tensor.matmul(
                            ps[0:rows, q * 512 : (q + 1) * 512],
                            lhsT,
                            lnx[:, q * 512 : (q + 1) * 512],
                        )

                def u_pair(which):
                    def emit():
                        ps = psum_s.tile([128, NC], f32, tag="ps",
                                         name=f"ps{which}{c}")
                        if which == "l":
                            mm2(ps, N_PAIR, g2_sb[:, 0:N_PAIR])
                            nc.scalar.activation(
                                pl[:, hcols], ps[0:N_PAIR, :], Act.Exp)
                        else:
                            mm2(ps, N_PAIR, g2_sb[:, N_PAIR : 2 * N_PAIR])
                            nc.scalar.activation(
                                pu[:, hcols], ps[0:N_PAIR, :], Act.Exp)
                    return emit

                def u_log(lb, r0):
                    def emit():
                        rows = LOG_ROWS[lb]
                        ps = psum_s.tile([128, NC], f32, tag="ps",
                                         name=f"pslog{c}_{lb}")
                        mm2(ps, rows, g3_sb[:, r0 : r0 + rows])
                        nc.scalar.activation(
                            slab[0:rows, N_MULT_BLK + lb, hcols],
                            ps[0:rows, :], Act.Exp)
                    return emit

                units = [u_pair("l"), u_pair("u")]
                r0 = 0
                for lb in range(N_LOG_BLK):
                    units.append(u_log(lb, r0))
                    r0 += LOG_ROWS[lb]
                return units

            def back_units(c):
                """(kind, emit) pair-gather+mult units of chunk c (DVE)."""
                m, h = divmod(c, MACRO // NC)
                pl, pu, slab = state[m]
                hcols = slice(h * NC, (h + 1) * NC)

                def u_mult(b):
                    def emit():
                        src_t = pl if b < 3 else pu
                        xrep = xrl_sbs[m] if b < 3 else xru_sbs[m]
                        o = (b % 3) * 128
                        ps_pg = psum_v.tile([128, NC], f32, tag="ps",
                                            name=f"pspg{c}_{b}")
                        for q in (0, 1):
                            nc.tensor.matmul(
                                ps_pg[:, q * 512 : (q + 1) * 512],
                                hp_sb[:, o : o + 128],
                                src_t[:, h * NC + q * 512
                                      : h * NC + (q + 1) * 512],
                            )
                        nc.vector.tensor_mul(
                            slab[:, b, hcols], ps_pg[:], xrep[:, hcols])
                    return emit

                return [u_mult(b) for b in range(N_MULT_BLK)]

            def emit_dmas(c, fine=False):
                m, h = divmod(c, MACRO // NC)
                pl, pu, slab = state[m]
                hcols = slice(h * NC, (h + 1) * NC)
                nc.sync.dma_start(
                    outT[m, ROW_SING : ROW_SING + 32, hcols],
                    xb_sbs[m][:, hcols])
                nc.sync.dma_start(
                    outT[m, ROW_PAIR_L : ROW_PAIR_L + N_PAIR, hcols],
                    pl[:, hcols])
                nc.sync.dma_start(
                    outT[m, ROW_PAIR_U : ROW_PAIR_U + N_PAIR, hcols],
                    pu[:, hcols])
                if not fine:
                    ot = outT.ap()[m, ROW_TRI : ROW_TRI + 8 * 128, hcols]
                    nc.gpsimd.dma_start(
                        ot.rearrange("(b p) c -> p b c", p=128),
                        slab[:, 0:8, hcols])
                    nc.gpsimd.dma_start(
                        outT[m, ROW_TRI + 8 * 128 : N_ROWS, hcols],
                        slab[0 : LOG_ROWS[2], 8, hcols])
                    return
                # per-block slab DMAs: each block streams out right after
                # its producer finishes -> short end-of-kernel drain
                for b in range(8):
                    nc.gpsimd.dma_start(
                        outT[m, ROW_TRI + b * 128 : ROW_TRI + (b + 1) * 128,
                             hcols],
                        slab[:, b, hcols])
                nc.gpsimd.dma_start(
                    outT[m, ROW_TRI + 8 * 128 : N_ROWS, hcols],
                    slab[0 : LOG_ROWS[2], 8, hcols])

            def interleave(f_units, b_units):
                out = []
                fi = bi = 0
                while fi < len(f_units) or bi < len(b_units):
                    if fi < len(f_units):
                        out.append(f_units[fi]); fi += 1
                    if bi < len(b_units):
                        out.append(b_units[bi]); bi += 1
                return out

            prev_back = []
            for c in range(N_CHUNK):
                for u in interleave(front_units(c), prev_back):
                    u()
                if c >= 1:
                    emit_dmas(c - 1)
                prev_back = back_units(c)
                if c % 2 == 1 and c // 2 + 1 < N_MACRO:
                    stage_inputs(c // 2 + 1)
            for u in prev_back:
                u()
            emit_dmas(N_CHUNK - 1, fine=True)

    nc.compile()
    return nc


def _spot_check(xl, xu, full_l, full_u, n_rows=48) -> bool:
    """Validate sampled rows against an exact host-side recomputation."""
    if not (np.isfinite(full_l).all() and np.isfinite(full_u).all()):
        return False
    rows = np.linspace(0, B_FULL - 1, n_rows, dtype=np.int64)
    idx2 = np.array(PAIRS)
    idx3 = np.array(TRIPLES)
    for x, out in ((xl, full_l), (xu, full_u)):
        xs = x[rows].astype(np.float64)
        exp = np.concatenate(
            [xs, np.prod(xs[:, idx2], -1), np.prod(xs[:, idx3], -1)], axis=1
        )
        rel = np.abs(out[rows] - exp) / np.maximum(np.abs(exp), 1e-9)
        if rel.max() > 1.5e-2:
            return False
    return True


def kernel(xl, xu):
    from concourse.bass_utils import run_bass_kernel_spmd

    xl = np.asarray(xl, dtype=np.float32)
    xu = np.asarray(xu, dtype=np.float32)

    if "nc" not in _CACHED:
        _CACHED["nc"] = _build_program()
    nc = _CACHED["nc"]

    import ml_dtypes

    kmap = np.concatenate(
        [np.full(cnt, k, dtype=np.int64) for k, cnt in K_ALLOC]
    )
    in_maps = []
    for i in range(N_CORES):
        lo, hi = i * B_CORE, (i + 1) * B_CORE
        xt = np.ascontiguousarray(
            np.concatenate([xl[lo:hi].T, xu[lo:hi].T], axis=0)
        )
        xbv = xt.astype(ml_dtypes.bfloat16)
        in_maps.append({
            "xt": xt,
            "xb": xbv,
            "xrl": np.ascontiguousarray(xbv[kmap]),
            "xru": np.ascontiguousarray(xbv[16 + kmap]),
        })

    *_, il, iu = _make_mats()
    # retry loop: guards against rare transient device/DMA corruption
    last_err = None
    full_l = full_u = None
    for attempt in range(3):
        try:
            res = run_bass_kernel_spmd(nc, in_maps, list(range(N_CORES)))
        except Exception as e:  # transient device error: retry
            last_err = e
            import time

            time.sleep(3)
            continue
        full_l = np.empty((B_FULL, N_OUT), dtype=np.float32)
        full_u = np.empty((B_FULL, N_OUT), dtype=np.float32)
        for i in range(N_CORES):
            lo, hi = i * B_CORE, (i + 1) * B_CORE
            ot = res.results[i]["outT"]            # [4, N_ROWS, MACRO]
            ot = ot.transpose(1, 0, 2).reshape(N_ROWS, B_CORE)
            full_l[lo:hi] = ot[il].T
            full_u[lo:hi] = ot[iu].T
        if _spot_check(xl, xu, full_l, full_u):
            return full_l, full_u
    if full_l is None:
        raise last_err
    return full_l, full_u
